# revision 1
# baseline (speedup 1.0000x reference)
"""Modulated deformable conv (DCNv2) Trainium2 Bass kernel.

Sharding: 8 cores = 4 batches x 2 pixel-halves (image rows 0-63 / 64-127).
Per core:
  A. Load padded image slice (fp16), conv weights, GEMM weights, base coords.
  B. Offset/mask convs as 9 shift-matmuls accumulated in PSUM -> [27, 8192] f32.
  C. PE-transpose conv out to pixel-partition layout; compute bilinear corner
     weights + gather indices exactly (clamp + validity masks, x-pair shift
     trick) with batched DVE ops; stage weight rows / index rows via DRAM.
  D. For each tap k (9) and y-row g (2): dma_gather (transpose mode) of
     x-row-pairs -> [128c, 2s, pix] fp16 tiles; multiply by partition-
     replicated weight maps; accumulate 36 matmuls [128c,128o]^T @ [128,pix]
     into PSUM -> out [128o, 8192] f32.
Pixel halves are disjoint; the host just concatenates the 8 outputs.
"""

import numpy as np

import concourse.bass as bass
import concourse.tile as tile
from concourse import bacc, mybir
from concourse.bass_utils import run_bass_kernel_spmd
from concourse.masks import make_identity

f16 = mybir.dt.float16
f32 = mybir.dt.float32
i16 = mybir.dt.int16
i32 = mybir.dt.int32
Alu = mybir.AluOpType
Act = mybir.ActivationFunctionType

H = W = 128
HW = H * W
C = 128
O = 128
K = 9
NCH = 27          # conv output channels: [off_y(9), off_x(9), mask_logit(9)]
NPX = HW // 2     # pixels per core (one half: 64 image rows)
BLK = NPX // 128  # 64 local row-blocks
CHUNK = 4096      # pixels per PSUM pass
NCHUNK = NPX // CHUNK  # 2
NS = NPX // 16    # index slots per row in dma_gather wrap layout


def _ap(src_ap, offset, pattern):
    """Raw AP at an element offset relative to an existing (DRAM) AP."""
    return bass.AP(tensor=src_ap.tensor, offset=src_ap.offset + offset,
                   ap=[list(p) for p in pattern])


def _apf(src_ap, offset, free_pattern):
    """SBUF/PSUM AP: keep the tile's partition dim, replace free dims."""
    return bass.AP(tensor=src_ap.tensor, offset=src_ap.offset + offset,
                   ap=[list(src_ap.ap[0])] + [list(p) for p in free_pattern])


def build_kernel(debug=False):
    nc = bacc.Bacc("TRN2", target_bir_lowering=False, debug=False,
                   enable_asserts=True)

    # ---- I/O (xpad is the 66 padded rows this core's half needs) ----
    xpad_in = nc.dram_tensor("xpad", [C, 66 * 130], f16, kind="ExternalInput")
    xrows_in = nc.dram_tensor("xrows", [HW * C], f16, kind="ExternalInput")
    wconv_in = nc.dram_tensor("wconv", [C, K * NCH], f16, kind="ExternalInput")
    bias_in = nc.dram_tensor("bias", [NCH, 1], f32, kind="ExternalInput")
    w2_in = nc.dram_tensor("w2", [C, K * O], f16, kind="ExternalInput")
    basey_in = nc.dram_tensor("basey", [128, K * BLK], f32, kind="ExternalInput")
    basex_in = nc.dram_tensor("basex", [128, K], f32, kind="ExternalInput")
    out_o = nc.dram_tensor("out", [O, NPX], f32, kind="ExternalOutput")

    wtrows_d = nc.dram_tensor("wtrows_d", [4 * K * NPX], f16)   # m = k*4+g*2+s
    idx_d = nc.dram_tensor("idx_d", [2 * K * NPX], i16)         # m2 = k*2+g

    if debug:
        dbg_conv = nc.dram_tensor("dbg_conv", [NCH, NPX], f32, kind="ExternalOutput")
        dbg_wt = nc.dram_tensor("dbg_wt", [4 * K, NPX], f16, kind="ExternalOutput")
        dbg_idx = nc.dram_tensor("dbg_idx", [2 * K, NPX], i16, kind="ExternalOutput")
        dbg_g = nc.dram_tensor("dbg_g", [128, 2 * CHUNK], f16, kind="ExternalOutput")

    with tile.TileContext(nc) as tc:
        with tc.tile_pool(name="persist", bufs=1) as persist:
            w2_t = persist.tile([C, K, O], f16)
            nc.sync.dma_start(w2_t[:], w2_in.ap())
            idx_sb = persist.tile([128, 2 * K, NS], i16)

            # ================= Phase B: offset/mask convs =================
            with tc.tile_pool(name="convph", bufs=1) as cph, \
                 tc.tile_pool(name="psconv", bufs=4, space="PSUM") as psc:
                xpad_t = cph.tile([C, 66, 130], f16)
                nc.sync.dma_start(xpad_t[:], xpad_in.ap())
                wconv_t = cph.tile([C, K, NCH], f16)
                nc.sync.dma_start(wconv_t[:], wconv_in.ap())
                bias_t = cph.tile([NCH, 1], f32)
                nc.sync.dma_start(bias_t[:], bias_in.ap())
                conv_sb = cph.tile([NCH, NPX], f32)
                ident = cph.tile([128, 128], f32)
                make_identity(nc, ident[:])

                for t in range(NPX // 512):  # 16 tiles of 512 px (4 rows)
                    ps = psc.tile([NCH, 512], f32)
                    for k in range(K):
                        ki, kj = k // 3, k % 3
                        rhs = _apf(xpad_t[:], (t * 4 + ki) * 130 + kj,
                                   [[130, 4], [1, 128]])
                        nc.tensor.matmul(ps[:], wconv_t[:, k, :], rhs,
                                         start=(k == 0), stop=(k == K - 1))
                    nc.scalar.activation(conv_sb[:, t * 512:(t + 1) * 512], ps[:],
                                         Act.Identity, bias=bias_t[:, 0:1])
                if debug:
                    nc.sync.dma_start(dbg_conv.ap(), conv_sb[:])

                # ========= Phase C: transpose + weight/index math =========
                with tc.tile_pool(name="wmath", bufs=1) as wm, \
                     tc.tile_pool(name="pst", bufs=2, space="PSUM") as pst:
                    offs = wm.tile([128, NCH, BLK], f32)
                    for grp in range(BLK // 16):
                        ps = pst.tile([128, 16 * NCH], f32)
                        for j in range(16):
                            blk = grp * 16 + j
                            nc.tensor.transpose(
                                ps[:, j * NCH:(j + 1) * NCH],
                                conv_sb[:, blk * 128:(blk + 1) * 128],
                                ident[0:NCH, 0:NCH])
                        src = _apf(ps[:], 0, [[1, NCH], [NCH, 16]])
                        dst = _apf(offs[:], grp * 16, [[BLK, NCH], [1, 16]])
                        nc.scalar.activation(dst, src, Act.Copy)

                    basey_t = wm.tile([128, K, BLK], f32)
                    nc.sync.dma_start(basey_t[:], basey_in.ap())
                    basex_t = wm.tile([128, K], f32)
                    nc.sync.dma_start(basex_t[:], basex_in.ap())

                    shp = [128, K, BLK]

                    def scratch(tag):
                        return wm.tile(shp, f32, tag=tag, name="sc_" + tag)

                    off_y = offs[:, 0:K, :]
                    off_x = offs[:, K:2 * K, :]
                    logits = offs[:, 2 * K:3 * K, :]

                    py = scratch("py")
                    nc.vector.tensor_tensor(py[:], off_y, basey_t[:], Alu.add)
                    px = scratch("px")
                    bx_b = basex_t[:, :, None].to_broadcast(tuple(shp))
                    nc.vector.tensor_tensor(px[:], off_x, bx_b, Alu.add)

                    def floor_(v, tag):
                        ri = wm.tile(shp, i32, tag="ri", name="ri")
                        nc.vector.tensor_copy(ri[:], v[:])
                        rf = scratch("rf")
                        nc.vector.tensor_copy(rf[:], ri[:])
                        gt = scratch("gt")
                        nc.vector.tensor_tensor(gt[:], rf[:], v[:], Alu.is_gt)
                        out = scratch(tag)
                        nc.vector.tensor_tensor(out[:], rf[:], gt[:], Alu.subtract)
                        return out

                    y0 = floor_(py, "y0")
                    x0 = floor_(px, "x0")
                    wy1 = scratch("wy1")
                    nc.vector.tensor_tensor(wy1[:], py[:], y0[:], Alu.subtract)
                    wx1 = scratch("wx1")
                    nc.vector.tensor_tensor(wx1[:], px[:], x0[:], Alu.subtract)
                    wy0 = scratch("wy0")
                    nc.vector.tensor_scalar(wy0[:], wy1[:], -1.0, 1.0, Alu.mult, Alu.add)
                    wx0 = scratch("wx0")
                    nc.vector.tensor_scalar(wx0[:], wx1[:], -1.0, 1.0, Alu.mult, Alu.add)

                    def in_range(v, lo, hi, tag):
                        a = scratch(tag)
                        b = scratch("rng")
                        nc.vector.tensor_scalar(a[:], v[:], float(lo), None, Alu.is_ge)
                        nc.vector.tensor_scalar(b[:], v[:], float(hi), None, Alu.is_le)
                        nc.vector.tensor_tensor(a[:], a[:], b[:], Alu.mult)
                        return a

                    vy0 = in_range(y0, 0, 127, "vy0")
                    vy1 = in_range(y0, -1, 126, "vy1")
                    vx0 = in_range(x0, 0, 127, "vx0")
                    vx1 = in_range(x0, -1, 126, "vx1")

                    msig = scratch("msig")
                    nc.scalar.activation(msig[:], logits, Act.Sigmoid)

                    # A_g = wy_g * vy_g * mask ; B_s = wx_s * vx_s
                    A0 = scratch("A0")
                    nc.vector.tensor_tensor(A0[:], wy0[:], vy0[:], Alu.mult)
                    nc.vector.tensor_tensor(A0[:], A0[:], msig[:], Alu.mult)
                    A1 = scratch("A1")
                    nc.vector.tensor_tensor(A1[:], wy1[:], vy1[:], Alu.mult)
                    nc.vector.tensor_tensor(A1[:], A1[:], msig[:], Alu.mult)
                    B0 = scratch("B0")
                    nc.vector.tensor_tensor(B0[:], wx0[:], vx0[:], Alu.mult)
                    B1 = scratch("B1")
                    nc.vector.tensor_tensor(B1[:], wx1[:], vx1[:], Alu.mult)

                    # x-pair base and slot weights
                    bx = scratch("bx")
                    nc.vector.tensor_scalar(bx[:], x0[:], 0.0, 126.0, Alu.max, Alu.min)
                    d = scratch("d")
                    nc.vector.tensor_tensor(d[:], x0[:], bx[:], Alu.subtract)
                    e0 = scratch("e0")
                    nc.vector.tensor_scalar(e0[:], d[:], 0.0, None, Alu.is_equal)
                    em = scratch("em")
                    nc.vector.tensor_scalar(em[:], d[:], -1.0, None, Alu.is_equal)
                    ep = scratch("ep")
                    nc.vector.tensor_scalar(ep[:], d[:], 1.0, None, Alu.is_equal)

                    ws0 = scratch("ws0")
                    t1 = scratch("t1")
                    nc.vector.tensor_tensor(ws0[:], B0[:], e0[:], Alu.mult)
                    nc.vector.tensor_tensor(t1[:], B1[:], em[:], Alu.mult)
                    nc.vector.tensor_tensor(ws0[:], ws0[:], t1[:], Alu.add)
                    ws1 = scratch("ws1")
                    t2 = scratch("t2")
                    nc.vector.tensor_tensor(ws1[:], B1[:], e0[:], Alu.mult)
                    nc.vector.tensor_tensor(t2[:], B0[:], ep[:], Alu.mult)
                    nc.vector.tensor_tensor(ws1[:], ws1[:], t2[:], Alu.add)

                    # weight maps -> f16, layout [128, (g,s):4, K, BLK]
                    wtpp = wm.tile([128, 4, K, BLK], f16)
                    for g, Ag in ((0, A0), (1, A1)):
                        for s, Ws in ((0, ws0), (1, ws1)):
                            nc.vector.tensor_tensor(wtpp[:, g * 2 + s, :, :],
                                                    Ag[:], Ws[:], Alu.mult)

                    # gather row indices: top = clamp(y0,0,127)*128+bx,
                    # bot = (clamp(y0,-1,126)+1)*128+bx
                    idxf = wm.tile([128, 2, K, BLK], f32)
                    yc = scratch("yc")
                    nc.vector.tensor_scalar(yc[:], y0[:], 0.0, 127.0, Alu.max, Alu.min)
                    nc.vector.tensor_scalar(yc[:], yc[:], 128.0, None, Alu.mult)
                    nc.vector.tensor_tensor(idxf[:, 0, :, :], yc[:], bx[:], Alu.add)
                    yc2 = scratch("yc2")
                    nc.vector.tensor_scalar(yc2[:], y0[:], -1.0, 126.0, Alu.max, Alu.min)
                    nc.vector.tensor_scalar(yc2[:], yc2[:], 1.0, 128.0, Alu.add, Alu.mult)
                    nc.vector.tensor_tensor(idxf[:, 1, :, :], yc2[:], bx[:], Alu.add)
                    idx16 = wm.tile([128, 2, K, BLK], i16)
                    nc.vector.tensor_copy(idx16[:], idxf[:])

                    # stage to DRAM in gather order i = pp*64 + blk (pp-major)
                    for m4 in range(4):
                        nc.sync.dma_start(
                            _ap(wtrows_d.ap(), m4 * NPX,
                                [[BLK, 128], [4 * NPX, K], [1, BLK]]),
                            wtpp[:, m4, :, :])
                    for m2 in range(2):
                        nc.sync.dma_start(
                            _ap(idx_d.ap(), m2 * NPX,
                                [[BLK, 128], [2 * NPX, K], [1, BLK]]),
                            idx16[:, m2, :, :])

            # wrap indices to dma_gather layout ([i%16, i//16]) with one xbar
            # transpose of [2K*NS, 16] -> [16, 2K*NS], then replicate x8 groups
            idx_tr = persist.tile([16, 2 * K * NS], i16)
            nc.sync.dma_start_transpose(
                idx_tr[:], _ap(idx_d.ap(), 0, [[16, 2 * K * NS], [1, 16]]))
            for g8 in range(8):
                nc.sync.dma_start(idx_sb[g8 * 16:(g8 + 1) * 16, :, :],
                                  idx_tr[:].rearrange("p (m s) -> p m s", m=2 * K))
            if debug:
                nc.sync.dma_start(dbg_wt.ap(),
                                  _ap(wtrows_d.ap(), 0, [[NPX, 4 * K], [1, NPX]]))
                nc.sync.dma_start(dbg_idx.ap(),
                                  _ap(idx_d.ap(), 0, [[NPX, 2 * K], [1, NPX]]))

            # ============ Phase D: gather + weight + GEMM ============
            with tc.tile_pool(name="gath", bufs=3) as gp, \
                 tc.tile_pool(name="wrep", bufs=3) as wp, \
                 tc.tile_pool(name="wgt", bufs=2) as wgp, \
                 tc.tile_pool(name="oev", bufs=2) as op_, \
                 tc.tile_pool(name="psout", bufs=1, space="PSUM") as pso:
                for ch in range(NCHUNK):
                    ps = pso.tile([O, CHUNK], f32)
                    for k in range(K):
                        for g in range(2):
                            m2 = k * 2 + g
                            gt = gp.tile([128, 2, CHUNK], f16, tag="g")
                            in_ap = _ap(xrows_in.ap(), 0,
                                        [[128, HW - 1], [1, 256]])
                            out_ap = _apf(gt[:], 0, [[CHUNK, 2], [1, CHUNK]])
                            idxs = idx_sb[:, m2,
                                          ch * (CHUNK // 16):(ch + 1) * (CHUNK // 16)]
                            nc.gpsimd.dma_gather(out_ap, in_ap, idxs,
                                                 num_idxs=CHUNK,
                                                 num_idxs_reg=CHUNK,
                                                 elem_size=256, elem_step=128,
                                                 transpose=True,
                                                 single_packet=False)
                            if debug and ch == 0 and k == 0 and g == 0:
                                nc.sync.dma_start(dbg_g.ap(), gt[:])
                            wr = wp.tile([128, 2, CHUNK], f16, tag="w")
                            nc.sync.dma_start(
                                wr[:],
                                _ap(wtrows_d.ap(),
                                    (k * 4 + g * 2) * NPX + ch * CHUNK,
                                    [[0, 128], [NPX, 2], [1, CHUNK]]))
                            wg = wgp.tile([128, 2, CHUNK], f16, tag="x")
                            nc.vector.tensor_tensor(wg[:], gt[:], wr[:], Alu.mult)
                            for s in range(2):
                                first = (k == 0 and g == 0 and s == 0)
                                last = (k == K - 1 and g == 1 and s == 1)
                                for b in range(CHUNK // 512):
                                    nc.tensor.matmul(
                                        ps[:, b * 512:(b + 1) * 512],
                                        w2_t[:, k, :],
                                        wg[:, s, b * 512:(b + 1) * 512],
                                        start=first, stop=last)
                    ot = op_.tile([O, CHUNK], f32, tag="o")
                    nc.scalar.activation(ot[:], ps[:], Act.Copy)
                    nc.sync.dma_start(
                        _ap(out_o.ap(), ch * CHUNK, [[NPX, O], [1, CHUNK]]), ot[:])
    nc.compile()
    return nc


def _host_inputs(x, w_off, b_off, w_mod, b_mod, w_reg):
    """Build the 8 per-core input maps."""
    # conv weights reordered: [off_y(9), off_x(9), mask(9)]
    wcat = np.concatenate([w_off[0::2], w_off[1::2], w_mod], axis=0)  # [27,128,3,3]
    bcat = np.concatenate([b_off[0::2], b_off[1::2], b_mod], axis=0)  # [27]
    wconv = np.ascontiguousarray(
        wcat.transpose(1, 2, 3, 0).reshape(C, K * NCH)).astype(np.float16)
    bias = bcat.reshape(NCH, 1).astype(np.float32)
    w2 = np.ascontiguousarray(
        (w_reg * 2.0).transpose(1, 2, 3, 0).reshape(C, K * O)).astype(np.float16)
    ki = np.arange(K) // 3
    kj = np.arange(K) % 3
    basex = (np.arange(128)[:, None] + kj[None, :] - 1).astype(np.float32)

    maps = []
    for core in range(8):
        b, hf = core // 2, core % 2
        xpadfull = np.zeros((C, 130, 130), dtype=np.float16)
        xpadfull[:, 1:129, 1:129] = x[b].astype(np.float16)
        xpad = np.ascontiguousarray(xpadfull[:, 64 * hf:64 * hf + 66, :])
        xrows = np.ascontiguousarray(
            x[b].transpose(1, 2, 0).reshape(HW * C)).astype(np.float16)
        rloc = 64 * hf + np.arange(BLK)
        basey = np.broadcast_to(
            (rloc[None, :] + ki[:, None] - 1)[None, :, :],
            (128, K, BLK)).reshape(128, K * BLK).astype(np.float32)
        maps.append({
            "xpad": xpad.reshape(C, 66 * 130),
            "xrows": xrows,
            "wconv": wconv,
            "bias": bias,
            "w2": w2,
            "basey": np.ascontiguousarray(basey),
            "basex": basex,
        })
    return maps


_NC_CACHE = {}


def kernel(x, w_off, b_off, w_mod, b_mod, w_reg, debug=False, trace=False):
    x = np.asarray(x)
    key = ("nc", debug)
    if key not in _NC_CACHE:
        _NC_CACHE[key] = build_kernel(debug=debug)
    nc = _NC_CACHE[key]
    maps = _host_inputs(x, np.asarray(w_off), np.asarray(b_off),
                        np.asarray(w_mod), np.asarray(b_mod), np.asarray(w_reg))
    res = run_bass_kernel_spmd(nc, maps, core_ids=list(range(8)), trace=trace)
    B = x.shape[0]
    out = np.empty((B, O, H, W), dtype=np.float32)
    for core in range(8):
        b, hf = core // 2, core % 2
        out[b, :, 64 * hf:64 * (hf + 1), :] = \
            res.results[core]["out"].reshape(O, 128, BLK).transpose(0, 2, 1)
    kernel._last_results = res
    return out



# revision 6
# speedup vs baseline: 1.7482x; 1.7482x over previous
"""Modulated deformable conv (DCNv2) Trainium2 Bass kernel.

Sharding: 8 cores = 4 batches x 2 pixel-halves (image rows 0-63 / 64-127).

Host prep (data-independent): xq[y*128+x] = corner quad
  [x[:,y,x], x[:,y,x+1], x[:,y+1,x], x[:,y+1,x+1]] -> [16384, 512] f16 per
  batch, so ONE gather index fetches all 4 bilinear corners of one tap.

Per core:
  B. Offset/mask convs as 9 shift-matmuls in PSUM -> [27, 8192] f32.
  C. PE-transpose conv out to pixel-partition layout [128pp, 27, 64blk];
     compute per-tap quad weights wt[128, 4q, 9k, 64blk] (f16, stays in
     SBUF) and quad indices idx = clamp(y0,0,126)*128 + clamp(x0,0,126)
     with slot-select weights handling the clamp; stage indices via DRAM
     into dma_gather wrap layout [128, 9k, 4ch, 128].
  D. For each chunk (4 x 2048 px) and tap k: non-transpose dma_gather of
     quads -> gt[128pp, 16slot, 512]; DVE-combine 4 corners with wt ->
     val[128pp, 16, 128c]; PE-transpose -> [128c, 2048px]; scalar-copy to
     SBUF f16; matmul w2 -> accumulate out PSUM [128o, 2048] over 9 taps.
Pixel halves are disjoint; the host just concatenates the 8 outputs.
"""

import numpy as np

import concourse.bass as bass
import concourse.tile as tile
from concourse import bacc, mybir
from concourse.bass_utils import run_bass_kernel_spmd
from concourse.masks import make_identity

f16 = mybir.dt.float16
f32 = mybir.dt.float32
i16 = mybir.dt.int16
i32 = mybir.dt.int32
Alu = mybir.AluOpType
Act = mybir.ActivationFunctionType

H = W = 128
HW = H * W
C = 128
O = 128
K = 9
NCH = 27          # conv output channels: [off_y(9), off_x(9), mask_logit(9)]
NPX = HW // 2     # pixels per core (one half: 64 image rows)
BLK = NPX // 128  # 64 local row-blocks
CHUNK = 2048      # pixels per PSUM pass
NCHUNK = NPX // CHUNK  # 4
SLOTS = CHUNK // 128   # 16 row-blocks per chunk


def _ap(src_ap, offset, pattern):
    """Raw AP at an element offset relative to an existing (DRAM) AP."""
    return bass.AP(tensor=src_ap.tensor, offset=src_ap.offset + offset,
                   ap=[list(p) for p in pattern])


def _apf(src_ap, offset, free_pattern):
    """SBUF/PSUM AP: keep the tile's partition dim, replace free dims."""
    return bass.AP(tensor=src_ap.tensor, offset=src_ap.offset + offset,
                   ap=[list(src_ap.ap[0])] + [list(p) for p in free_pattern])


def build_kernel(debug=False):
    nc = bacc.Bacc("TRN2", target_bir_lowering=False, debug=False,
                   enable_asserts=True)

    # ---- I/O ----
    xq_in = nc.dram_tensor("xq", [HW * 4 * C], f16, kind="ExternalInput")
    xpad_in = nc.dram_tensor("xpad", [C, 66 * 130], f16, kind="ExternalInput")
    wconv_in = nc.dram_tensor("wconv", [C, K * NCH], f16, kind="ExternalInput")
    bias_in = nc.dram_tensor("bias", [NCH, 1], f32, kind="ExternalInput")
    w2_in = nc.dram_tensor("w2", [C, K * O], f16, kind="ExternalInput")
    basey_in = nc.dram_tensor("basey", [128, K * BLK], f32, kind="ExternalInput")
    basex_in = nc.dram_tensor("basex", [128, K], f32, kind="ExternalInput")
    out_o = nc.dram_tensor("out", [O, NPX], f32, kind="ExternalOutput")

    idx_d = nc.dram_tensor("idx_d", [128 * K * BLK], i16)  # [pp, k, blk]

    if debug:
        dbg_conv = nc.dram_tensor("dbg_conv", [NCH, NPX], f32, kind="ExternalOutput")
        dbg_wt = nc.dram_tensor("dbg_wt", [128, 4 * K * BLK], f16, kind="ExternalOutput")
        dbg_idx = nc.dram_tensor("dbg_idx", [128, K * BLK], i16, kind="ExternalOutput")
        dbg_g = nc.dram_tensor("dbg_g", [128, SLOTS * 512], f16, kind="ExternalOutput")
        dbg_val = nc.dram_tensor("dbg_val", [128, SLOTS * 128], f16, kind="ExternalOutput")

    with tile.TileContext(nc) as tc:
        with tc.tile_pool(name="persist", bufs=1) as persist:
            w2_t = persist.tile([C, K, O], f16)
            nc.sync.dma_start(w2_t[:], w2_in.ap())
            wt_t = persist.tile([128, 4, K, BLK], f16)
            idx_sb = persist.tile([128, K, NCHUNK * 128], i16)
            identf = persist.tile([128, 128], f16)
            make_identity(nc, identf[:])

            # ================= Phase B: offset/mask convs =================
            with tc.tile_pool(name="convph", bufs=1) as cph, \
                 tc.tile_pool(name="psconv", bufs=4, space="PSUM") as psc:
                xpad_t = cph.tile([C, 66, 130], f16)
                nc.sync.dma_start(xpad_t[:], xpad_in.ap())
                wconv_t = cph.tile([C, K, NCH], f16)
                nc.sync.dma_start(wconv_t[:], wconv_in.ap())
                bias_t = cph.tile([NCH, 1], f32)
                nc.sync.dma_start(bias_t[:], bias_in.ap())
                conv_sb = cph.tile([NCH, NPX], f32)
                ident = cph.tile([128, 128], f32)
                make_identity(nc, ident[:])

                for t in range(NPX // 512):  # 16 tiles of 512 px (4 rows)
                    ps = psc.tile([NCH, 512], f32)
                    for k in range(K):
                        ki, kj = k // 3, k % 3
                        rhs = _apf(xpad_t[:], (t * 4 + ki) * 130 + kj,
                                   [[130, 4], [1, 128]])
                        nc.tensor.matmul(ps[:], wconv_t[:, k, :], rhs,
                                         start=(k == 0), stop=(k == K - 1))
                    nc.scalar.activation(conv_sb[:, t * 512:(t + 1) * 512], ps[:],
                                         Act.Identity, bias=bias_t[:, 0:1])
                if debug:
                    nc.sync.dma_start(dbg_conv.ap(), conv_sb[:])

                # ========= Phase C: transpose + weight/index math =========
                with tc.tile_pool(name="wmath", bufs=1) as wm, \
                     tc.tile_pool(name="pst", bufs=2, space="PSUM") as pst:
                    offs = wm.tile([128, NCH, BLK], f32)
                    for grp in range(BLK // 16):
                        ps = pst.tile([128, 16 * NCH], f32)
                        for j in range(16):
                            blk = grp * 16 + j
                            nc.tensor.transpose(
                                ps[:, j * NCH:(j + 1) * NCH],
                                conv_sb[:, blk * 128:(blk + 1) * 128],
                                ident[0:NCH, 0:NCH])
                        src = _apf(ps[:], 0, [[1, NCH], [NCH, 16]])
                        dst = _apf(offs[:], grp * 16, [[BLK, NCH], [1, 16]])
                        nc.scalar.activation(dst, src, Act.Copy)

                    basey_t = wm.tile([128, K, BLK], f32)
                    nc.sync.dma_start(basey_t[:], basey_in.ap())
                    basex_t = wm.tile([128, K], f32)
                    nc.sync.dma_start(basex_t[:], basex_in.ap())

                    shp = [128, K, BLK]

                    def scratch(tag):
                        return wm.tile(shp, f32, tag=tag, name="sc_" + tag)

                    off_y = offs[:, 0:K, :]
                    off_x = offs[:, K:2 * K, :]
                    logits = offs[:, 2 * K:3 * K, :]

                    py = scratch("py")
                    nc.vector.tensor_tensor(py[:], off_y, basey_t[:], Alu.add)
                    px = scratch("px")
                    bx_b = basex_t[:, :, None].to_broadcast(tuple(shp))
                    nc.vector.tensor_tensor(px[:], off_x, bx_b, Alu.add)

                    def floor_(v, tag):
                        ri = wm.tile(shp, i32, tag="ri", name="ri")
                        nc.vector.tensor_copy(ri[:], v[:])
                        rf = scratch("rf")
                        nc.vector.tensor_copy(rf[:], ri[:])
                        gt = scratch("gt")
                        nc.vector.tensor_tensor(gt[:], rf[:], v[:], Alu.is_gt)
                        out = scratch(tag)
                        nc.vector.tensor_tensor(out[:], rf[:], gt[:], Alu.subtract)
                        return out

                    y0 = floor_(py, "y0")
                    x0 = floor_(px, "x0")
                    wy1 = scratch("wy1")
                    nc.vector.tensor_tensor(wy1[:], py[:], y0[:], Alu.subtract)
                    wx1 = scratch("wx1")
                    nc.vector.tensor_tensor(wx1[:], px[:], x0[:], Alu.subtract)
                    wy0 = scratch("wy0")
                    nc.vector.tensor_scalar(wy0[:], wy1[:], -1.0, 1.0, Alu.mult, Alu.add)
                    wx0 = scratch("wx0")
                    nc.vector.tensor_scalar(wx0[:], wx1[:], -1.0, 1.0, Alu.mult, Alu.add)

                    def in_range(v, lo, hi, tag):
                        a = scratch(tag)
                        b = scratch("rng")
                        nc.vector.tensor_scalar(a[:], v[:], float(lo), None, Alu.is_ge)
                        nc.vector.tensor_scalar(b[:], v[:], float(hi), None, Alu.is_le)
                        nc.vector.tensor_tensor(a[:], a[:], b[:], Alu.mult)
                        return a

                    vy0 = in_range(y0, 0, 127, "vy0")
                    vy1 = in_range(y0, -1, 126, "vy1")
                    vx0 = in_range(x0, 0, 127, "vx0")
                    vx1 = in_range(x0, -1, 126, "vx1")

                    msig = scratch("msig")
                    nc.scalar.activation(msig[:], logits, Act.Sigmoid)

                    # A_g = wy_g * vy_g * mask ; B_s = wx_s * vx_s
                    A0 = scratch("A0")
                    nc.vector.tensor_tensor(A0[:], wy0[:], vy0[:], Alu.mult)
                    nc.vector.tensor_tensor(A0[:], A0[:], msig[:], Alu.mult)
                    A1 = scratch("A1")
                    nc.vector.tensor_tensor(A1[:], wy1[:], vy1[:], Alu.mult)
                    nc.vector.tensor_tensor(A1[:], A1[:], msig[:], Alu.mult)
                    B0 = scratch("B0")
                    nc.vector.tensor_tensor(B0[:], wx0[:], vx0[:], Alu.mult)
                    B1 = scratch("B1")
                    nc.vector.tensor_tensor(B1[:], wx1[:], vx1[:], Alu.mult)

                    # slot-select weights for a clamped base b = clamp(v0,0,126):
                    # slot0 covers row b (corner v0 iff d==0, corner v0+1 iff d==-1)
                    # slot1 covers row b+1 (corner v0+1 iff d==0, corner v0 iff d==1)
                    def slot_weights(v0, W0, W1, tag):
                        b = scratch("b" + tag)
                        nc.vector.tensor_scalar(b[:], v0[:], 0.0, 126.0, Alu.max, Alu.min)
                        d = scratch("d" + tag)
                        nc.vector.tensor_tensor(d[:], v0[:], b[:], Alu.subtract)
                        e0 = scratch("e0" + tag)
                        nc.vector.tensor_scalar(e0[:], d[:], 0.0, None, Alu.is_equal)
                        em = scratch("em" + tag)
                        nc.vector.tensor_scalar(em[:], d[:], -1.0, None, Alu.is_equal)
                        ep = scratch("ep" + tag)
                        nc.vector.tensor_scalar(ep[:], d[:], 1.0, None, Alu.is_equal)
                        ws0 = scratch("ws0" + tag)
                        t1 = scratch("t1" + tag)
                        nc.vector.tensor_tensor(ws0[:], W0[:], e0[:], Alu.mult)
                        nc.vector.tensor_tensor(t1[:], W1[:], em[:], Alu.mult)
                        nc.vector.tensor_tensor(ws0[:], ws0[:], t1[:], Alu.add)
                        ws1 = scratch("ws1" + tag)
                        t2 = scratch("t2" + tag)
                        nc.vector.tensor_tensor(ws1[:], W1[:], e0[:], Alu.mult)
                        nc.vector.tensor_tensor(t2[:], W0[:], ep[:], Alu.mult)
                        nc.vector.tensor_tensor(ws1[:], ws1[:], t2[:], Alu.add)
                        return b, ws0, ws1

                    by, wsy0, wsy1 = slot_weights(y0, A0, A1, "y")
                    bx, wsx0, wsx1 = slot_weights(x0, B0, B1, "x")

                    # quad weights wt[q=2*sy+sx] = wsy_sy * wsx_sx  -> f16
                    for sy, Wy in ((0, wsy0), (1, wsy1)):
                        for sx, Wx in ((0, wsx0), (1, wsx1)):
                            nc.vector.tensor_tensor(wt_t[:, sy * 2 + sx, :, :],
                                                    Wy[:], Wx[:], Alu.mult)

                    # quad index = by*128 + bx
                    idxf = scratch("idxf")
                    nc.vector.tensor_scalar(idxf[:], by[:], 128.0, None, Alu.mult)
                    nc.vector.tensor_tensor(idxf[:], idxf[:], bx[:], Alu.add)
                    idx16 = wm.tile(shp, i16, tag="idx16", name="idx16")
                    nc.vector.tensor_copy(idx16[:], idxf[:])

                    # stage indices to DRAM [pp, k, blk], reload wrapped:
                    # dst[16p, k, blk, h] <- dram[(h*16+p)*576 + k*64 + blk]
                    nc.sync.dma_start(
                        _ap(idx_d.ap(), 0, [[K * BLK, 128], [BLK, K], [1, BLK]]),
                        idx16[:])
                    if debug:
                        nc.sync.dma_start(dbg_wt.ap(), wt_t[:])
                        nc.sync.dma_start(dbg_idx.ap(), idx16[:])

            idx_tr = persist.tile([16, K, BLK, 8], i16)
            nc.sync.dma_start(
                idx_tr[:],
                _ap(idx_d.ap(), 0,
                    [[K * BLK, 16], [BLK, K], [1, BLK], [16 * K * BLK, 8]]))
            for g8 in range(8):
                nc.sync.dma_start(
                    idx_sb[g8 * 16:(g8 + 1) * 16, :, :],
                    idx_tr[:].rearrange("p k b h -> p k (b h)"))

            # ============ Phase D: gather + combine + GEMM ============
            with tc.tile_pool(name="gath", bufs=3) as gp, \
                 tc.tile_pool(name="vp", bufs=2) as vp, \
                 tc.tile_pool(name="vtp", bufs=2) as vtp, \
                 tc.tile_pool(name="oev", bufs=2) as op_, \
                 tc.tile_pool(name="pstr", bufs=1, space="PSUM") as pstr, \
                 tc.tile_pool(name="psout", bufs=1, space="PSUM") as pso:
                for ch in range(NCHUNK):
                    out_ps = pso.tile([O, CHUNK], f32)
                    for k in range(K):
                        gt = gp.tile([128, SLOTS, 512], f16, tag="g")
                        in_ap = _ap(xq_in.ap(), 0, [[512, HW], [1, 512]])
                        out_ap = _apf(gt[:], 0, [[512, SLOTS], [1, 512]])
                        nc.gpsimd.dma_gather(out_ap, in_ap,
                                             idx_sb[:, k, ch * 128:(ch + 1) * 128],
                                             num_idxs=CHUNK, num_idxs_reg=CHUNK,
                                             elem_size=512, elem_step=512,
                                             transpose=False,
                                             single_packet=False)
                        if debug and ch == 0 and k == 0:
                            nc.sync.dma_start(dbg_g.ap(), gt[:])
                        val = vp.tile([128, SLOTS, 128], f16, tag="v")
                        tmp = vp.tile([128, SLOTS, 128], f16, tag="t")
                        wslice = wt_t[:, :, k, ch * SLOTS:(ch + 1) * SLOTS]
                        for q in range(4):
                            wb = wslice[:, q, :][:, :, None].to_broadcast(
                                (128, SLOTS, 128))
                            dstq = val if q == 0 else tmp
                            nc.vector.tensor_tensor(
                                dstq[:], gt[:, :, q * 128:(q + 1) * 128], wb,
                                Alu.mult)
                            if q > 0:
                                nc.vector.tensor_tensor(val[:], val[:], tmp[:],
                                                        Alu.add)
                        if debug and ch == 0 and k == 0:
                            nc.sync.dma_start(dbg_val.ap(), val[:])
                        psT = pstr.tile([128, CHUNK], f16)
                        for j in range(SLOTS):
                            nc.tensor.transpose(psT[:, j * 128:(j + 1) * 128],
                                                val[:, j, :], identf[:])
                        valT = vtp.tile([128, CHUNK], f16, tag="vt")
                        nc.scalar.activation(valT[:], psT[:], Act.Copy)
                        for b in range(CHUNK // 512):
                            nc.tensor.matmul(
                                out_ps[:, b * 512:(b + 1) * 512],
                                w2_t[:, k, :],
                                valT[:, b * 512:(b + 1) * 512],
                                start=(k == 0), stop=(k == K - 1))
                    ot = op_.tile([O, CHUNK], f32, tag="o")
                    nc.scalar.activation(ot[:], out_ps[:], Act.Copy)
                    nc.sync.dma_start(
                        _ap(out_o.ap(), ch * CHUNK, [[NPX, O], [1, CHUNK]]),
                        ot[:])
    nc.compile()
    return nc


def _host_inputs(x, w_off, b_off, w_mod, b_mod, w_reg):
    """Build the 8 per-core input maps."""
    # conv weights reordered: [off_y(9), off_x(9), mask(9)]
    wcat = np.concatenate([w_off[0::2], w_off[1::2], w_mod], axis=0)  # [27,128,3,3]
    bcat = np.concatenate([b_off[0::2], b_off[1::2], b_mod], axis=0)  # [27]
    wconv = np.ascontiguousarray(
        wcat.transpose(1, 2, 3, 0).reshape(C, K * NCH)).astype(np.float16)
    bias = bcat.reshape(NCH, 1).astype(np.float32)
    w2 = np.ascontiguousarray(
        (w_reg * 2.0).transpose(1, 2, 3, 0).reshape(C, K * O)).astype(np.float16)
    ki = np.arange(K) // 3
    kj = np.arange(K) % 3
    basex = (np.arange(128)[:, None] + kj[None, :] - 1).astype(np.float32)

    # corner-quad layout per batch: xq[y*128+x] = [x(y,x), x(y,x+1),
    # x(y+1,x), x(y+1,x+1)] channels-contiguous
    B = x.shape[0]
    xf = x.astype(np.float16)
    xq_all = []
    for b in range(B):
        xp = np.zeros((129, 129, C), dtype=np.float16)
        xp[:128, :128] = xf[b].transpose(1, 2, 0)
        quad = np.empty((128, 128, 4, C), dtype=np.float16)
        quad[:, :, 0] = xp[:128, :128]
        quad[:, :, 1] = xp[:128, 1:129]
        quad[:, :, 2] = xp[1:129, :128]
        quad[:, :, 3] = xp[1:129, 1:129]
        xq_all.append(np.ascontiguousarray(quad.reshape(HW * 4 * C)))

    maps = []
    for core in range(8):
        b, hf = core // 2, core % 2
        xpadfull = np.zeros((C, 130, 130), dtype=np.float16)
        xpadfull[:, 1:129, 1:129] = xf[b]
        xpad = np.ascontiguousarray(xpadfull[:, 64 * hf:64 * hf + 66, :])
        rloc = 64 * hf + np.arange(BLK)
        basey = np.broadcast_to(
            (rloc[None, :] + ki[:, None] - 1)[None, :, :],
            (128, K, BLK)).reshape(128, K * BLK).astype(np.float32)
        maps.append({
            "xq": xq_all[b],
            "xpad": xpad.reshape(C, 66 * 130),
            "wconv": wconv,
            "bias": bias,
            "w2": w2,
            "basey": np.ascontiguousarray(basey),
            "basex": basex,
        })
    return maps


_NC_CACHE = {}


def kernel(x, w_off, b_off, w_mod, b_mod, w_reg, debug=False, trace=False):
    x = np.asarray(x)
    key = ("nc", debug)
    if key not in _NC_CACHE:
        _NC_CACHE[key] = build_kernel(debug=debug)
    nc = _NC_CACHE[key]
    maps = _host_inputs(x, np.asarray(w_off), np.asarray(b_off),
                        np.asarray(w_mod), np.asarray(b_mod), np.asarray(w_reg))
    res = run_bass_kernel_spmd(nc, maps, core_ids=list(range(8)), trace=trace)
    B = x.shape[0]
    out = np.empty((B, O, H, W), dtype=np.float32)
    for core in range(8):
        b, hf = core // 2, core % 2
        out[b, :, 64 * hf:64 * (hf + 1), :] = \
            res.results[core]["out"].reshape(O, BLK, 128)
    kernel._last_results = res
    return out


# revision 13
# speedup vs baseline: 2.5758x; 1.4735x over previous
"""Modulated deformable conv (DCNv2) Trainium2 Bass kernel.

Sharding: 8 cores = 4 batches x 2 pixel-halves (image rows 0-63 / 64-127).

Host prep (data-independent): xq[y*128+x] = corner quad
  [x[:,y,x], x[:,y,x+1], x[:,y+1,x], x[:,y+1,x+1]] -> [16384, 512] f16 per
  batch, so ONE gather index fetches all 4 bilinear corners of one tap.

Per core:
  B. Offset/mask convs as 9 shift-matmuls in PSUM -> [27, 8192] f32.
  C. PE-transpose conv out to pixel-partition layout [128pp, 27, 64blk];
     compute per-tap quad weights wt[128, 4q, 9k, 64blk] (f16, stays in
     SBUF) and quad indices idx = clamp(y0,0,126)*128 + clamp(x0,0,126)
     with slot-select weights handling the clamp; stage indices via DRAM
     into dma_gather wrap layout [128, 9k, 4ch, 128].
  D. For each chunk (4 x 2048 px) and tap k: non-transpose dma_gather of
     quads -> gt[128pp, 16slot, 512]; DVE-combine 4 corners with wt ->
     val[128pp, 16, 128c]; PE-transpose -> [128c, 2048px]; scalar-copy to
     SBUF f16; matmul w2 -> accumulate out PSUM [128o, 2048] over 9 taps.
Pixel halves are disjoint; the host just concatenates the 8 outputs.
"""

import numpy as np

import concourse.bass as bass
import concourse.tile as tile
from concourse import bacc, mybir
from concourse.bass_utils import run_bass_kernel_spmd
from concourse.masks import make_identity

f16 = mybir.dt.float16
f32 = mybir.dt.float32
i16 = mybir.dt.int16
i32 = mybir.dt.int32
Alu = mybir.AluOpType
Act = mybir.ActivationFunctionType

H = W = 128
HW = H * W
C = 128
O = 128
K = 9
NCH = 27          # conv output channels: [off_y(9), off_x(9), mask_logit(9)]
NPX = HW // 2     # pixels per core (one half: 64 image rows)
BLK = NPX // 128  # 64 local row-blocks
CHUNK = 2048      # pixels per PSUM pass
NCHUNK = NPX // CHUNK  # 4
SLOTS = CHUNK // 128   # 16 row-blocks per chunk


def _ap(src_ap, offset, pattern):
    """Raw AP at an element offset relative to an existing (DRAM) AP."""
    return bass.AP(tensor=src_ap.tensor, offset=src_ap.offset + offset,
                   ap=[list(p) for p in pattern])


def _apf(src_ap, offset, free_pattern):
    """SBUF/PSUM AP: keep the tile's partition dim, replace free dims."""
    return bass.AP(tensor=src_ap.tensor, offset=src_ap.offset + offset,
                   ap=[list(src_ap.ap[0])] + [list(p) for p in free_pattern])


def build_kernel(debug=False):
    nc = bacc.Bacc("TRN2", target_bir_lowering=False, debug=False,
                   enable_asserts=True, dynamic_dma_scratch_size=32768)

    # ---- I/O ----
    xq_in = nc.dram_tensor("xq", [HW * 4 * C], f16, kind="ExternalInput")
    xpad_in = nc.dram_tensor("xpad", [C, 66 * 130], f16, kind="ExternalInput")
    wconv_in = nc.dram_tensor("wconv", [C, K * NCH], f16, kind="ExternalInput")
    bias_in = nc.dram_tensor("bias", [NCH, 1], f32, kind="ExternalInput")
    w2_in = nc.dram_tensor("w2", [C, K * O], f16, kind="ExternalInput")
    basey_in = nc.dram_tensor("basey", [128, K * BLK], f32, kind="ExternalInput")
    basex_in = nc.dram_tensor("basex", [128, K], f32, kind="ExternalInput")
    out_o = nc.dram_tensor("out", [O, NPX], f32, kind="ExternalOutput")

    idx_d = nc.dram_tensor("idx_d", [128 * K * BLK], i16)  # [pp, k, blk]

    if debug:
        dbg_conv = nc.dram_tensor("dbg_conv", [NCH, NPX], f32, kind="ExternalOutput")
        dbg_wt = nc.dram_tensor("dbg_wt", [128, K * BLK * 4], f16, kind="ExternalOutput")
        dbg_idx = nc.dram_tensor("dbg_idx", [128, K * BLK], i16, kind="ExternalOutput")
        dbg_g = nc.dram_tensor("dbg_g", [128, SLOTS * 512], f16, kind="ExternalOutput")
        dbg_val = nc.dram_tensor("dbg_val", [128, SLOTS * 128], f32, kind="ExternalOutput")

    with tile.TileContext(nc) as tc:
        with tc.tile_pool(name="persist", bufs=1) as persist:
            w2_t = persist.tile([C, K, O], f16)
            nc.sync.dma_start(w2_t[:], w2_in.ap())
            wt_t = persist.tile([128, K, BLK, 4], f16)
            idx_sb = persist.tile([128, K, NCHUNK * 128], i16)
            identp = persist.tile([128, 128], f32)
            make_identity(nc, identp[:])

            # ================= Phase B: offset/mask convs =================
            with tc.tile_pool(name="convph", bufs=1) as cph, \
                 tc.tile_pool(name="psconv", bufs=4, space="PSUM") as psc:
                xpad_t = cph.tile([C, 66, 130], f16)
                nc.sync.dma_start(xpad_t[:], xpad_in.ap())
                wconv_t = cph.tile([C, K, NCH], f16)
                nc.sync.dma_start(wconv_t[:], wconv_in.ap())
                bias_t = cph.tile([NCH, 1], f32)
                nc.sync.dma_start(bias_t[:], bias_in.ap())
                conv_sb = cph.tile([NCH, NPX], f32)
                ident = cph.tile([128, 128], f32)
                make_identity(nc, ident[:])

                for t in range(NPX // 512):  # 16 tiles of 512 px (4 rows)
                    ps = psc.tile([NCH, 512], f32)
                    for k in range(K):
                        ki, kj = k // 3, k % 3
                        rhs = _apf(xpad_t[:], (t * 4 + ki) * 130 + kj,
                                   [[130, 4], [1, 128]])
                        nc.tensor.matmul(ps[:], wconv_t[:, k, :], rhs,
                                         start=(k == 0), stop=(k == K - 1))
                    nc.scalar.activation(conv_sb[:, t * 512:(t + 1) * 512], ps[:],
                                         Act.Identity, bias=bias_t[:, 0:1])
                if debug:
                    nc.sync.dma_start(dbg_conv.ap(), conv_sb[:])

                # ========= Phase C: transpose + weight/index math =========
                with tc.tile_pool(name="wmath", bufs=1) as wm, \
                     tc.tile_pool(name="pst", bufs=2, space="PSUM") as pst:
                    offs = wm.tile([128, NCH, BLK], f32)
                    for grp in range(BLK // 16):
                        ps = pst.tile([128, 16 * NCH], f32)
                        for j in range(16):
                            blk = grp * 16 + j
                            nc.tensor.transpose(
                                ps[:, j * NCH:(j + 1) * NCH],
                                conv_sb[:, blk * 128:(blk + 1) * 128],
                                ident[0:NCH, 0:NCH])
                        src = _apf(ps[:], 0, [[1, NCH], [NCH, 16]])
                        dst = _apf(offs[:], grp * 16, [[BLK, NCH], [1, 16]])
                        nc.scalar.activation(dst, src, Act.Copy)

                    basey_t = wm.tile([128, K, BLK], f32)
                    nc.sync.dma_start(basey_t[:], basey_in.ap())
                    basex_t = wm.tile([128, K], f32)
                    nc.sync.dma_start(basex_t[:], basex_in.ap())

                    shp = [128, K, BLK]

                    def scratch(tag):
                        return wm.tile(shp, f32, tag=tag, name="sc_" + tag)

                    off_y = offs[:, 0:K, :]
                    off_x = offs[:, K:2 * K, :]
                    logits = offs[:, 2 * K:3 * K, :]

                    py = scratch("py")
                    nc.vector.tensor_tensor(py[:], off_y, basey_t[:], Alu.add)
                    px = scratch("px")
                    bx_b = basex_t[:, :, None].to_broadcast(tuple(shp))
                    nc.vector.tensor_tensor(px[:], off_x, bx_b, Alu.add)

                    def floor_(v, tag):
                        ri = wm.tile(shp, i32, tag="ri", name="ri")
                        nc.vector.tensor_copy(ri[:], v[:])
                        rf = scratch("rf")
                        nc.vector.tensor_copy(rf[:], ri[:])
                        gt = scratch("gt")
                        nc.vector.tensor_tensor(gt[:], rf[:], v[:], Alu.is_gt)
                        out = scratch(tag)
                        nc.vector.tensor_tensor(out[:], rf[:], gt[:], Alu.subtract)
                        return out

                    y0 = floor_(py, "y0")
                    x0 = floor_(px, "x0")
                    wy1 = scratch("wy1")
                    nc.vector.tensor_tensor(wy1[:], py[:], y0[:], Alu.subtract)
                    wx1 = scratch("wx1")
                    nc.vector.tensor_tensor(wx1[:], px[:], x0[:], Alu.subtract)
                    wy0 = scratch("wy0")
                    nc.vector.tensor_scalar(wy0[:], wy1[:], -1.0, 1.0, Alu.mult, Alu.add)
                    wx0 = scratch("wx0")
                    nc.vector.tensor_scalar(wx0[:], wx1[:], -1.0, 1.0, Alu.mult, Alu.add)

                    def in_range(v, lo, hi, tag):
                        a = scratch(tag)
                        b = scratch("rng")
                        nc.vector.tensor_scalar(a[:], v[:], float(lo), None, Alu.is_ge)
                        nc.vector.tensor_scalar(b[:], v[:], float(hi), None, Alu.is_le)
                        nc.vector.tensor_tensor(a[:], a[:], b[:], Alu.mult)
                        return a

                    vy0 = in_range(y0, 0, 127, "vy0")
                    vy1 = in_range(y0, -1, 126, "vy1")
                    vx0 = in_range(x0, 0, 127, "vx0")
                    vx1 = in_range(x0, -1, 126, "vx1")

                    msig = scratch("msig")
                    nc.scalar.activation(msig[:], logits, Act.Sigmoid)

                    # A_g = wy_g * vy_g * mask ; B_s = wx_s * vx_s
                    A0 = scratch("A0")
                    nc.vector.tensor_tensor(A0[:], wy0[:], vy0[:], Alu.mult)
                    nc.vector.tensor_tensor(A0[:], A0[:], msig[:], Alu.mult)
                    A1 = scratch("A1")
                    nc.vector.tensor_tensor(A1[:], wy1[:], vy1[:], Alu.mult)
                    nc.vector.tensor_tensor(A1[:], A1[:], msig[:], Alu.mult)
                    B0 = scratch("B0")
                    nc.vector.tensor_tensor(B0[:], wx0[:], vx0[:], Alu.mult)
                    B1 = scratch("B1")
                    nc.vector.tensor_tensor(B1[:], wx1[:], vx1[:], Alu.mult)

                    # slot-select weights for a clamped base b = clamp(v0,0,126):
                    # slot0 covers row b (corner v0 iff d==0, corner v0+1 iff d==-1)
                    # slot1 covers row b+1 (corner v0+1 iff d==0, corner v0 iff d==1)
                    def slot_weights(v0, W0, W1, tag):
                        b = scratch("b" + tag)
                        nc.vector.tensor_scalar(b[:], v0[:], 0.0, 126.0, Alu.max, Alu.min)
                        d = scratch("d" + tag)
                        nc.vector.tensor_tensor(d[:], v0[:], b[:], Alu.subtract)
                        e0 = scratch("e0" + tag)
                        nc.vector.tensor_scalar(e0[:], d[:], 0.0, None, Alu.is_equal)
                        em = scratch("em" + tag)
                        nc.vector.tensor_scalar(em[:], d[:], -1.0, None, Alu.is_equal)
                        ep = scratch("ep" + tag)
                        nc.vector.tensor_scalar(ep[:], d[:], 1.0, None, Alu.is_equal)
                        ws0 = scratch("ws0" + tag)
                        t1 = scratch("t1" + tag)
                        nc.vector.tensor_tensor(ws0[:], W0[:], e0[:], Alu.mult)
                        nc.vector.tensor_tensor(t1[:], W1[:], em[:], Alu.mult)
                        nc.vector.tensor_tensor(ws0[:], ws0[:], t1[:], Alu.add)
                        ws1 = scratch("ws1" + tag)
                        t2 = scratch("t2" + tag)
                        nc.vector.tensor_tensor(ws1[:], W1[:], e0[:], Alu.mult)
                        nc.vector.tensor_tensor(t2[:], W0[:], ep[:], Alu.mult)
                        nc.vector.tensor_tensor(ws1[:], ws1[:], t2[:], Alu.add)
                        return b, ws0, ws1

                    by, wsy0, wsy1 = slot_weights(y0, A0, A1, "y")
                    bx, wsx0, wsx1 = slot_weights(x0, B0, B1, "x")

                    # quad weights wt[..., q=2*sy+sx] = wsy_sy * wsx_sx  (f16,
                    # q innermost so the Phase D combine runs in DVE 2x mode)
                    for sy, Wy in ((0, wsy0), (1, wsy1)):
                        for sx, Wx in ((0, wsx0), (1, wsx1)):
                            nc.vector.tensor_tensor(wt_t[:, :, :, sy * 2 + sx],
                                                    Wy[:], Wx[:], Alu.mult)

                    # quad index = by*128 + bx
                    idxf = scratch("idxf")
                    nc.vector.tensor_scalar(idxf[:], by[:], 128.0, None, Alu.mult)
                    nc.vector.tensor_tensor(idxf[:], idxf[:], bx[:], Alu.add)
                    idx16 = wm.tile(shp, i16, tag="idx16", name="idx16")
                    nc.vector.tensor_copy(idx16[:], idxf[:])

                    # stage indices to DRAM [pp, k, blk], reload wrapped:
                    # dst[16p, k, blk, h] <- dram[(h*16+p)*576 + k*64 + blk]
                    nc.sync.dma_start(
                        _ap(idx_d.ap(), 0, [[K * BLK, 128], [BLK, K], [1, BLK]]),
                        idx16[:])
                    if debug:
                        nc.sync.dma_start(dbg_wt.ap(), wt_t[:])
                        nc.sync.dma_start(dbg_idx.ap(), idx16[:])

            # reload wrapped: dst[16p, k, h, blk] <- dram[(h*16+p)*576+k*64+blk]
            # (blk innermost keeps the DMA in contiguous 128B runs), then a
            # DVE copy permutes to gather order [16p, k, (ch, slot, h)] with
            # j = slot*128 + h*16 + p = slot*128 + pp.
            idx_raw = persist.tile([16, K, 8, BLK], i16)
            nc.sync.dma_start(
                idx_raw[:],
                _ap(idx_d.ap(), 0,
                    [[K * BLK, 16], [BLK, K], [16 * K * BLK, 8], [1, BLK]]))
            idx_tr = persist.tile([16, K, NCHUNK, 16, 8], i16)
            for k in range(K):
                src = bass.AP(tensor=idx_raw[:].tensor,
                              offset=idx_raw[:].offset + k * 8 * BLK,
                              ap=[list(idx_raw[:].ap[0]),
                                  [16, NCHUNK], [1, 16], [BLK, 8]])
                nc.vector.tensor_copy(idx_tr[:, k, :, :, :], src)
            for g8 in range(8):
                nc.sync.dma_start(
                    idx_sb[g8 * 16:(g8 + 1) * 16, :, :],
                    idx_tr[:].rearrange("p k c s h -> p k (c s h)"))

            # ============ Phase D: gather + combine + GEMM ============
            with tc.tile_pool(name="gath", bufs=3) as gp, \
                 tc.tile_pool(name="vp", bufs=2) as vp, \
                 tc.tile_pool(name="vtp", bufs=2) as vtp, \
                 tc.tile_pool(name="oev", bufs=2) as op_, \
                 tc.tile_pool(name="pstr", bufs=1, space="PSUM") as pstr, \
                 tc.tile_pool(name="psout", bufs=1, space="PSUM") as pso:
                for ch in range(NCHUNK):
                    out_ps = pso.tile([O, CHUNK], f32)
                    for k in range(K):
                        gt = gp.tile([128, SLOTS, 512], f16, tag="g")
                        in_ap = _ap(xq_in.ap(), 0, [[512, HW], [1, 512]])
                        out_ap = _apf(gt[:], 0, [[512, SLOTS], [1, 512]])
                        nc.gpsimd.dma_gather(out_ap, in_ap,
                                             idx_sb[:, k, ch * 128:(ch + 1) * 128],
                                             num_idxs=CHUNK, num_idxs_reg=CHUNK,
                                             elem_size=512, elem_step=512,
                                             transpose=False,
                                             single_packet=False)
                        if debug and ch == 0 and k == 0:
                            nc.sync.dma_start(dbg_g.ap(), gt[:])
                        # gt element layout is [c, q] (q innermost): one 2x-mode
                        # multiply by the broadcast quad weights, then an
                        # innermost-axis add-reduce over q.
                        tmp = vp.tile([128, SLOTS, 128, 4], f16, tag="t")
                        val = vp.tile([128, SLOTS, 128], f32, tag="v")
                        wb = wt_t[:, k, ch * SLOTS:(ch + 1) * SLOTS, :][
                            :, :, None, :].to_broadcast((128, SLOTS, 128, 4))
                        gq = _apf(gt[:], 0, [[512, SLOTS], [4, 128], [1, 4]])
                        nc.vector.tensor_tensor(tmp[:], gq, wb, Alu.mult)
                        nc.vector.tensor_reduce(val[:], tmp[:],
                                                mybir.AxisListType.X, Alu.add)
                        if debug and ch == 0 and k == 0:
                            nc.sync.dma_start(dbg_val.ap(), val[:])
                        psT = pstr.tile([128, CHUNK], f32)
                        for j in range(SLOTS):
                            nc.tensor.transpose(psT[:, j * 128:(j + 1) * 128],
                                                val[:, j, :], identp[:])
                        valT = vtp.tile([128, CHUNK], f16, tag="vt")
                        nc.scalar.activation(valT[:], psT[:], Act.Copy)
                        for b in range(CHUNK // 512):
                            nc.tensor.matmul(
                                out_ps[:, b * 512:(b + 1) * 512],
                                w2_t[:, k, :],
                                valT[:, b * 512:(b + 1) * 512],
                                start=(k == 0), stop=(k == K - 1))
                    ot = op_.tile([O, CHUNK], f32, tag="o")
                    nc.scalar.activation(ot[:], out_ps[:], Act.Copy)
                    nc.sync.dma_start(
                        _ap(out_o.ap(), ch * CHUNK, [[NPX, O], [1, CHUNK]]),
                        ot[:])
    nc.compile()
    return nc


def _host_inputs(x, w_off, b_off, w_mod, b_mod, w_reg):
    """Build the 8 per-core input maps."""
    # conv weights reordered: [off_y(9), off_x(9), mask(9)]
    wcat = np.concatenate([w_off[0::2], w_off[1::2], w_mod], axis=0)  # [27,128,3,3]
    bcat = np.concatenate([b_off[0::2], b_off[1::2], b_mod], axis=0)  # [27]
    wconv = np.ascontiguousarray(
        wcat.transpose(1, 2, 3, 0).reshape(C, K * NCH)).astype(np.float16)
    bias = bcat.reshape(NCH, 1).astype(np.float32)
    w2 = np.ascontiguousarray(
        (w_reg * 2.0).transpose(1, 2, 3, 0).reshape(C, K * O)).astype(np.float16)
    ki = np.arange(K) // 3
    kj = np.arange(K) % 3
    basex = (np.arange(128)[:, None] + kj[None, :] - 1).astype(np.float32)

    # corner-quad layout per batch, q innermost: xq[y*128+x][c][q] with
    # q = [x(y,x), x(y,x+1), x(y+1,x), x(y+1,x+1)][c]
    B = x.shape[0]
    xf = x.astype(np.float16)
    xq_all = []
    for b in range(B):
        xp = np.zeros((129, 129, C), dtype=np.float16)
        xp[:128, :128] = xf[b].transpose(1, 2, 0)
        quad = np.stack([xp[:128, :128], xp[:128, 1:129],
                         xp[1:129, :128], xp[1:129, 1:129]], axis=-1)
        xq_all.append(np.ascontiguousarray(quad.reshape(HW * 4 * C)))

    maps = []
    for core in range(8):
        b, hf = core // 2, core % 2
        xpadfull = np.zeros((C, 130, 130), dtype=np.float16)
        xpadfull[:, 1:129, 1:129] = xf[b]
        xpad = np.ascontiguousarray(xpadfull[:, 64 * hf:64 * hf + 66, :])
        rloc = 64 * hf + np.arange(BLK)
        basey = np.broadcast_to(
            (rloc[None, :] + ki[:, None] - 1)[None, :, :],
            (128, K, BLK)).reshape(128, K * BLK).astype(np.float32)
        maps.append({
            "xq": xq_all[b],
            "xpad": xpad.reshape(C, 66 * 130),
            "wconv": wconv,
            "bias": bias,
            "w2": w2,
            "basey": np.ascontiguousarray(basey),
            "basex": basex,
        })
    return maps


_NC_CACHE = {}


def kernel(x, w_off, b_off, w_mod, b_mod, w_reg, debug=False, trace=False):
    x = np.asarray(x)
    key = ("nc", debug)
    if key not in _NC_CACHE:
        _NC_CACHE[key] = build_kernel(debug=debug)
    nc = _NC_CACHE[key]
    maps = _host_inputs(x, np.asarray(w_off), np.asarray(b_off),
                        np.asarray(w_mod), np.asarray(b_mod), np.asarray(w_reg))
    res = run_bass_kernel_spmd(nc, maps, core_ids=list(range(8)), trace=trace)
    B = x.shape[0]
    out = np.empty((B, O, H, W), dtype=np.float32)
    for core in range(8):
        b, hf = core // 2, core % 2
        out[b, :, 64 * hf:64 * (hf + 1), :] = \
            res.results[core]["out"].reshape(O, BLK, 128)
    kernel._last_results = res
    return out
